# revision 2
# baseline (speedup 1.0000x reference)
"""Trainium2 Bass kernel for ComplexSpectralAttention.

Math note: with q = [q_r|q_i] (128 dims per head), Re(Q K^H) = q_r.k_r + q_i.k_i
is just the full 128-dim dot product q.k, and [out_r|out_i] = probs @ [v_r|v_i].
So this is standard 16-head causal attention with head_dim 128 and scale
1/sqrt(64), followed by the Wo projection.

Sharding (8 cores): 2-way data parallel over batch x 4-way tensor parallel over
heads. Core c handles batch b=c//4 and heads [4g, 4g+4) where g=c%4.

Per-core schedule (engines in FIFO emission order, so ordering is the
performance contract):
  - V projection, then per head: k/q projection (query chunks descending),
    then attention per 512-query chunk (descending, so later normalize
    latency hides under bigger subsequent tiles).
  - Attention inner loop is software-pipelined: the S matmuls + exp of group
    g+1 are emitted BEFORE the PV matmuls of group g, so PE never sits
    waiting for ACT's exp. Diagonal key blocks compute only the causal
    query subrange (memset-zeroed remainder) with a [128,128] triangular
    mask on the leading block.
  - Softmax denominators: DVE accumulates per-key-block partials (bf16),
    GpSimd partition_all_reduce collapses the 128 partitions (idle engine,
    no PSUM bank, no DRAM roundtrip), DVE reciprocal + fused
    normalize-and-copy from PSUM into outT. The reciprocal+mul is deferred
    by one tile so the allreduce latency hides behind the next tile's work.
  - Wo partial product per 128-token chunk, descending so the last head's
    final normalize only gates the last-emitted chunks.
Host sums the 4 partials per batch. All matmuls bf16 with fp32 accumulate.
"""

import numpy as np
import ml_dtypes

B, N, C = 2, 2048, 1024
GC = 512          # per-core head columns (4 heads x 128)
HPC = 4           # heads per core
KC = 8            # contraction chunks of 128 over C
NT = N // 512     # 4 query chunks of 512
NKB = N // 128    # 16 key blocks of 128

_CACHE = {}

_BF16 = ml_dtypes.bfloat16


def _build_nc(repeat=1):
    import concourse.bacc as bacc
    import concourse.mybir as mybir
    import concourse.tile as tile
    from concourse import bass_isa

    f32 = mybir.dt.float32
    bf16 = mybir.dt.bfloat16
    Exp = mybir.ActivationFunctionType.Exp
    RAdd = bass_isa.ReduceOp.add

    nc = bacc.Bacc("TRN2", target_bir_lowering=False, debug=False, num_devices=8)
    xt_d = nc.dram_tensor("xt", [C, N], bf16, kind="ExternalInput").ap()
    wq_d = nc.dram_tensor("wq", [C, GC], bf16, kind="ExternalInput").ap()
    wk_d = nc.dram_tensor("wk", [C, GC], bf16, kind="ExternalInput").ap()
    wv_d = nc.dram_tensor("wv", [C, GC], bf16, kind="ExternalInput").ap()
    wo_d = nc.dram_tensor("wo", [GC, C], bf16, kind="ExternalInput").ap()
    mask_d = nc.dram_tensor("mask", [128, 128], bf16, kind="ExternalInput").ap()
    out_d = nc.dram_tensor("out", [N, C], f32, kind="ExternalOutput").ap()

    with tile.TileContext(nc) as tc:
        with tc.tile_pool(name="const", bufs=1) as const:
            xt_sb = const.tile([128, KC, N], bf16, name="xt_sb")
            xt_r = xt_d.rearrange("(a p) n -> p a n", p=128)
            for kc in range(KC):
                nc.sync.dma_start(out=xt_sb[:, kc, :], in_=xt_r[:, kc, :])
            wv_sb = const.tile([128, KC, GC], bf16, name="wv_sb")
            wv_r = wv_d.rearrange("(a p) n -> p a n", p=128)
            for kc in range(KC):
                nc.sync.dma_start(out=wv_sb[:, kc, :], in_=wv_r[:, kc, :])
            wq_sb = const.tile([128, KC, GC], bf16, name="wq_sb")
            nc.sync.dma_start(out=wq_sb, in_=wq_d.rearrange("(a p) n -> p a n", p=128))
            wk_sb = const.tile([128, KC, GC], bf16, name="wk_sb")
            nc.sync.dma_start(out=wk_sb, in_=wk_d.rearrange("(a p) n -> p a n", p=128))
            wo_sb = const.tile([128, HPC, C], bf16, name="wo_sb")
            nc.sync.dma_start(out=wo_sb, in_=wo_d.rearrange("(h p) n -> p h n", p=128))
            mask_sb = const.tile([128, 128], bf16, name="mask_sb")
            nc.sync.dma_start(out=mask_sb, in_=mask_d)

            qt_sb = const.tile([128, HPC, N], bf16, name="qt_sb")
            kt_sb = const.tile([128, HPC, N], bf16, name="kt_sb")
            v_sb = const.tile([128, NKB, GC], bf16, name="v_sb")
            outT_sb = const.tile([128, HPC, N], bf16, name="outT_sb")

            def emit_body():
                # PSUM banks: ps1(2x1) + psS(2x2) + psO(2x1) = 8.
                with (
                    tc.tile_pool(name="ps1", bufs=2, space="PSUM") as ps1,
                    tc.tile_pool(name="psS", bufs=2, space="PSUM") as psS,
                    tc.tile_pool(name="psO", bufs=2, space="PSUM") as psO,
                    tc.tile_pool(name="ptp", bufs=3) as ptp,
                    tc.tile_pool(name="accp", bufs=2) as accp,
                    tc.tile_pool(name="nrm", bufs=2) as nrm,
                ):
                    # v projection: v[tok, d] tiles feed all heads
                    for t in range(NKB):
                        acc = ps1.tile([128, 512], f32, tag="proj", name="acc")
                        for kc in range(KC):
                            nc.tensor.matmul(
                                acc,
                                xt_sb[:, kc, t * 128 : (t + 1) * 128],
                                wv_sb[:, kc, :],
                                start=(kc == 0),
                                stop=(kc == KC - 1),
                            )
                        nc.vector.tensor_copy(v_sb[:, t, :], acc)

                    # deferred per-(h,t) normalize: reciprocal of the
                    # allreduced denominator + fused normalize/copy of the
                    # PSUM attention accumulator into outT
                    pending = [None]

                    def flush_norm():
                        if pending[0] is not None:
                            ar, acc_o, h_, t_ = pending[0]
                            pending[0] = None
                            rden = nrm.tile([128, 512], f32, tag="rden", name="rden")
                            nc.vector.reciprocal_approx_fast(rden, ar)
                            nc.vector.tensor_mul(
                                outT_sb[:, h_, t_ * 512 : (t_ + 1) * 512], acc_o, rden
                            )

                    for h in range(HPC):
                        # k^T then q^T projection, chunks descending (attention
                        # starts at t=3 and needs all of kt but only qt chunk 3)
                        for wsb, dst in ((wk_sb, kt_sb), (wq_sb, qt_sb)):
                            for t in range(NT - 1, -1, -1):
                                acc = ps1.tile([128, 512], f32, tag="proj", name="acc")
                                for kc in range(KC):
                                    nc.tensor.matmul(
                                        acc,
                                        wsb[:, kc, h * 128 : (h + 1) * 128],
                                        xt_sb[:, kc, t * 512 : (t + 1) * 512],
                                        start=(kc == 0),
                                        stop=(kc == KC - 1),
                                    )
                                nc.vector.tensor_copy(
                                    dst[:, h, t * 512 : (t + 1) * 512], acc
                                )

                        for t in range(NT - 1, -1, -1):
                            nkb = 4 * t + 4  # key blocks in play (causal)
                            acc_o = psO.tile([128, 512], f32, tag="acc_o", name="acc_o")
                            psum = accp.tile([128, 512], bf16, tag="psum", name="psum")
                            groups = [(kb0, False) for kb0 in range(0, 4 * t, 2)]
                            groups += [(4 * t, True), (4 * t + 2, True)]
                            prev = None
                            first_psum = True

                            def emit_pv(pgroup):
                                pp, pkb0 = pgroup
                                for j in range(2):
                                    kb = pkb0 + j
                                    nc.tensor.matmul(
                                        acc_o,
                                        v_sb[:, kb, h * 128 : (h + 1) * 128],
                                        pp[:, j * 512 : (j + 1) * 512],
                                        start=(kb == 0),
                                        stop=(kb == nkb - 1),
                                    )

                            for gi, (kb0, diag) in enumerate(groups):
                                s = psS.tile([128, 1024], f32, tag="s", name="s")
                                p = ptp.tile([128, 1024], bf16, tag="p", name="p")
                                for j in range(2):
                                    kb = kb0 + j
                                    c0 = j * 512 + ((kb - 4 * t) * 128 if diag else 0)
                                    q0 = t * 512 + (c0 - j * 512)
                                    nc.tensor.matmul(
                                        s[:, c0 : (j + 1) * 512],
                                        kt_sb[:, h, kb * 128 : (kb + 1) * 128],
                                        qt_sb[:, h, q0 : (t + 1) * 512],
                                        start=True,
                                        stop=True,
                                    )
                                if not diag:
                                    nc.scalar.activation(p, s, Exp, scale=0.125)
                                    if first_psum:
                                        nc.vector.tensor_copy(psum, p[:, 0:512])
                                        first_psum = False
                                    else:
                                        nc.vector.tensor_add(psum, psum, p[:, 0:512])
                                    nc.vector.tensor_add(psum, psum, p[:, 512:1024])
                                else:
                                    for j in range(2):
                                        kb = kb0 + j
                                        jj = kb - 4 * t
                                        c0 = j * 512 + jj * 128
                                        if jj > 0:
                                            # zero the causally-dead region so
                                            # the full-width PV matmul is safe
                                            nc.vector.memset(p[:, j * 512 : c0], 0.0)
                                        nc.scalar.activation(
                                            p[:, c0 : (j + 1) * 512],
                                            s[:, c0 : (j + 1) * 512],
                                            Exp,
                                            scale=0.125,
                                        )
                                        nc.vector.tensor_mul(
                                            p[:, c0 : c0 + 128],
                                            p[:, c0 : c0 + 128],
                                            mask_sb,
                                        )
                                        if first_psum:
                                            nc.vector.tensor_copy(psum, p[:, 0:512])
                                            first_psum = False
                                        else:
                                            nc.vector.tensor_add(
                                                psum[:, jj * 128 : 512],
                                                psum[:, jj * 128 : 512],
                                                p[:, c0 : (j + 1) * 512],
                                            )
                                if gi == 0:
                                    # previous tile's normalize lands here so
                                    # its allreduce latency stayed hidden
                                    flush_norm()
                                if prev is not None:
                                    emit_pv(prev)
                                prev = (p, kb0)
                            emit_pv(prev)
                            # denominator: collapse partitions on GpSimd
                            ar = nrm.tile([128, 512], f32, tag="ar", name="ar")
                            nc.gpsimd.partition_all_reduce(ar, psum, 128, RAdd)
                            pending[0] = (ar, acc_o, h, t)
                    flush_norm()

                # ---- Wo partial product -----------------------------------
                with (
                    tc.tile_pool(name="psF", bufs=2, space="PSUM") as psF,
                    tc.tile_pool(name="fout", bufs=3) as fpool,
                ):
                    for t in range(NKB - 1, -1, -1):  # 16 chunks of 128 tokens
                        acc = psF.tile([128, C], f32, name="acc")
                        for n2 in range(2):
                            for h in range(HPC):
                                nc.tensor.matmul(
                                    acc[:, n2 * 512 : (n2 + 1) * 512],
                                    outT_sb[:, h, t * 128 : (t + 1) * 128],
                                    wo_sb[:, h, n2 * 512 : (n2 + 1) * 512],
                                    start=(h == 0),
                                    stop=(h == HPC - 1),
                                )
                        fo = fpool.tile([128, C], f32, name="fo")
                        nc.scalar.copy(fo, acc)
                        nc.sync.dma_start(out=out_d[t * 128 : (t + 1) * 128, :], in_=fo)

            for _rep in range(repeat):
                emit_body()

    nc.compile()
    return nc


def _get_nc():
    if "nc" not in _CACHE:
        _CACHE["nc"] = _build_nc()
    return _CACHE["nc"]


def _make_mask():
    # triangular [128,128]: mask[i, q] = 1.0 iff q >= i (query index within
    # the leading 128-query window of a diagonal key block)
    i = np.arange(128)[:, None]
    qv = np.arange(128)[None, :]
    return (qv >= i).astype(_BF16)


def make_in_maps(x, Wq, Wk, Wv, Wo):
    mask = _make_mask()
    in_maps = []
    for c in range(8):
        b, g = divmod(c, 4)
        in_maps.append(
            {
                "xt": np.ascontiguousarray(x[b].T).astype(_BF16),
                "wq": np.ascontiguousarray(Wq[:, g * GC : (g + 1) * GC]).astype(_BF16),
                "wk": np.ascontiguousarray(Wk[:, g * GC : (g + 1) * GC]).astype(_BF16),
                "wv": np.ascontiguousarray(Wv[:, g * GC : (g + 1) * GC]).astype(_BF16),
                "wo": np.ascontiguousarray(Wo[g * GC : (g + 1) * GC, :]).astype(_BF16),
                "mask": mask,
            }
        )
    return in_maps


def gather_out(results):
    out = np.zeros((B, N, C), np.float32)
    for c in range(8):
        out[c // 4] += results[c]["out"]
    return out


def kernel(x, Wq, Wk, Wv, Wo):
    from concourse.bass_utils import run_bass_kernel_spmd

    nc = _get_nc()
    in_maps = make_in_maps(
        np.asarray(x, np.float32),
        np.asarray(Wq, np.float32),
        np.asarray(Wk, np.float32),
        np.asarray(Wv, np.float32),
        np.asarray(Wo, np.float32),
    )
    res = run_bass_kernel_spmd(nc, in_maps, core_ids=list(range(8)))
    return gather_out(res.results)
